# revision 1
# baseline (speedup 1.0000x reference)
"""Trainium2 Bass kernel for nn_LogOddsPerformanceTransformer.

Computes, for each element x of Xs:
    s   = log(x) - log(1-x)              (log-odds)
    idx = clip(searchsorted(bins, max(s, bins[0]), 'right') - 1, 0, NB-1)
    out = bins[idx]

bins is a uniform grid (linspace), so binning reduces to an affine floor
done entirely with fused 2-op vector instructions via the magic-number
rounding trick (no fmod, no gather, no division).  With
b0 = bins[0], step = (bins[-1]-bins[0])/(NB-1), inv = 1/step:

    t1 = s*inv + (2^23 + off)        off = -b0*inv - 0.5 (exact for these bins)
    t2 = clip(t1, 2^23, 2^23 + NB-1) # adding 2^23 floors s*inv+off to an int
    t3 = (t2 - 2^22) - (2^22 - b0*inv)   # both subtractions exact -> idx + b0*inv
    out = t3 * step                      # == idx*step + b0 up to 1 ulp

Data parallel over 8 NeuronCores; per core the 524288-element slice is
processed as a sequence of [128 x fsz] tiles (small head/tail tiles to
shorten pipeline ramp and drain).  t2/t3 instructions are greedily
balanced between the vector engine and gpsimd; the final scale always
runs on gpsimd so output DMAs never stall a compute sequencer.
"""

import sys

sys.path.insert(0, "/opt/trn_rl_repo")

from contextlib import ExitStack

import numpy as np

import concourse.bass as bass
import concourse.tile as tile
from concourse import bacc, mybir
from concourse.bass_utils import run_bass_kernel_spmd

N = 4_194_304
NCORES = 8
NPER = N // NCORES  # 524288
P = 128

# --- tunables -------------------------------------------------------------
TILE_SIZES = (256, 512, 512, 1024, 1024, 512, 256)  # sum = 4096
FC = 512  # DVE/pool compute chunk within a tile
ACT_FULL_TILE = False  # Ln at full tile size (fewer, bigger ACT instrs)
T3_POOL_PATTERN = (0, 1)  # cyclic: 1 -> chunk's unbias TS runs on gpsimd
T2_POOL_PATTERN = (0, 0, 0, 1)  # cyclic: 1 -> chunk's clamp TS runs on gpsimd
TT_POOL_PATTERN = (0,)  # cyclic: 1 -> chunk's subtract runs on gpsimd
FINAL_DVE_TAIL = 2  # last k chunks run t3+final on DVE (pool drains the tail)
LAST_OUT_POOL = False  # last chunk: final on pool + pool-issued out DMA (no sem hop)
DMA_IN_ENGINE = "sync"
DMA_OUT_ENGINE = "sync"
OUT_PER_CHUNK = True  # one out DMA per compute chunk instead of per tile
OPOOL_BUFS = 5
TMP_BUFS = 5
# --------------------------------------------------------------------------

f32 = mybir.dt.float32
Alu = mybir.AluOpType
Act = mybir.ActivationFunctionType

_BUILD_CACHE: dict[tuple, object] = {}


def _constants(bins: np.ndarray):
    """Host-side constants; returns None if the fused-exact path can't be
    used for these bins (non-uniform or inexact magic offsets)."""
    b64 = bins.astype(np.float64)
    nb = len(bins)
    step = np.float32((b64[-1] - b64[0]) / (nb - 1))
    inv = np.float32((nb - 1) / (b64[-1] - b64[0]))
    off = np.float32(-b64[0] * (nb - 1) / (b64[-1] - b64[0]) - 0.5)
    M = np.float32(2.0**23)
    C = np.float32(np.float64(M) + np.float64(off))
    M63 = np.float32(np.float64(M) + (nb - 1))
    U2 = np.float32(2.0**22)
    U2b = np.float32(2.0**22 + (np.float64(off) + 0.5))
    uniform = np.allclose(
        np.diff(b64), (b64[-1] - b64[0]) / (nb - 1), rtol=0, atol=1e-5
    )
    exact = (
        float(C) - float(M) == float(off)
        and float(U2b) == 2.0**22 + float(off) + 0.5
        and float(M63) == 2.0**23 + (nb - 1)
    )
    if not (uniform and exact):
        return None
    return tuple(float(v) for v in (step, inv, C, M, M63, U2, U2b))


# rough per-instruction cost estimates (ns) used only for load balancing
def _dve_ts(n):
    return (n / 2 + 58) / 0.96


def _pool_ts(n):
    return 1579.0 * n / 1024.0


def _build(step, inv, C, M, M63, U2, U2b):
    assert sum(TILE_SIZES) * P == NPER
    NT = len(TILE_SIZES)
    nc = bacc.Bacc("TRN2", target_bir_lowering=False, debug=False)
    xs = [
        nc.dram_tensor(f"xs{i}", [P, fsz], f32, kind="ExternalInput").ap()
        for i, fsz in enumerate(TILE_SIZES)
    ]
    outs = [
        nc.dram_tensor(f"out{i}", [P, fsz], f32, kind="ExternalOutput").ap()
        for i, fsz in enumerate(TILE_SIZES)
    ]

    with tile.TileContext(nc) as tc, ExitStack() as ctx:
        # bufs is per-tag: each x{i} tag is used exactly once, so 1 buf each
        xpool = ctx.enter_context(tc.tile_pool(name="xpool", bufs=1))
        opool = ctx.enter_context(tc.tile_pool(name="opool", bufs=OPOOL_BUFS))
        tmp = ctx.enter_context(tc.tile_pool(name="tmp", bufs=TMP_BUFS))
        dma_in = getattr(nc, DMA_IN_ENGINE)
        dma_out = getattr(nc, DMA_OUT_ENGINE)

        # all input DMAs issued first (high priority) so the out DMAs --
        # which block their sequencer until compute finishes -- never
        # starve later input tiles
        x_tiles = []
        with tc.high_priority():
            for i, fsz in enumerate(TILE_SIZES):
                x = xpool.tile([P, fsz], f32, tag=f"x{i}")
                dma_in.dma_start(x[:], xs[i][:])
                x_tiles.append(x)

        total_chunks = sum(-(-fsz // FC) for fsz in TILE_SIZES)
        chunk_idx = 0
        for i, fsz in enumerate(TILE_SIZES):
            x = x_tiles[i]
            o = opool.tile([P, fsz], f32, tag="o")
            a = tmp.tile([P, fsz], f32, tag="a")
            b = tmp.tile([P, fsz], f32, tag="b")
            if ACT_FULL_TILE:
                nc.scalar.activation(a[:], x[:], Act.Ln)
                nc.scalar.activation(b[:], x[:], Act.Ln, 1.0, -1.0)
            else:
                off = 0
                while off < fsz:
                    fa = min(FC, fsz - off)
                    sla = (slice(None), slice(off, off + fa))
                    nc.scalar.activation(a[sla], x[sla], Act.Ln)
                    nc.scalar.activation(b[sla], x[sla], Act.Ln, 1.0, -1.0)
                    off += fa
            off = 0
            while off < fsz:
                fc = min(FC, fsz - off)
                sl = (slice(None), slice(off, off + fc))
                s = tmp.tile([P, fc], f32, tag="s")
                tt_eng = (
                    nc.gpsimd
                    if TT_POOL_PATTERN[chunk_idx % len(TT_POOL_PATTERN)]
                    else nc.vector
                )
                tt_eng.tensor_sub(s[:], a[sl], b[sl])
                t1 = tmp.tile([P, fc], f32, tag="t1")
                nc.vector.tensor_scalar(t1[:], s[:], inv, C, Alu.mult, Alu.add)
                tail = chunk_idx >= total_chunks - FINAL_DVE_TAIL
                t2 = tmp.tile([P, fc], f32, tag="t2")
                t2_eng = (
                    nc.gpsimd
                    if (T2_POOL_PATTERN[chunk_idx % len(T2_POOL_PATTERN)] and not tail)
                    else nc.vector
                )
                t2_eng.tensor_scalar(t2[:], t1[:], M, M63, Alu.max, Alu.min)
                t3 = tmp.tile([P, fc], f32, tag="t3")
                t3_eng = (
                    nc.gpsimd
                    if (T3_POOL_PATTERN[chunk_idx % len(T3_POOL_PATTERN)] and not tail)
                    else nc.vector
                )
                last = chunk_idx == total_chunks - 1
                chunk_idx += 1
                t3_eng.tensor_scalar(t3[:], t2[:], U2, U2b, Alu.subtract, Alu.subtract)
                if last and LAST_OUT_POOL:
                    final_eng = nc.gpsimd
                elif tail:
                    final_eng = nc.vector
                else:
                    final_eng = nc.gpsimd
                final_eng.tensor_scalar(o[sl], t3[:], step, None, Alu.mult)
                if OUT_PER_CHUNK:
                    eng = nc.gpsimd if (last and LAST_OUT_POOL) else dma_out
                    eng.dma_start(outs[i][sl], o[sl])
                off += fc
            if not OUT_PER_CHUNK:
                dma_out.dma_start(outs[i][:], o[:])

    nc.compile()
    return nc


def build(bins: np.ndarray):
    key = _constants(bins)
    if key is None:
        raise NotImplementedError("non-uniform bins not supported by this kernel")
    if key not in _BUILD_CACHE:
        _BUILD_CACHE[key] = _build(*key)
    return _BUILD_CACHE[key]


def make_in_maps(Xs: np.ndarray):
    shards = Xs.reshape(NCORES, NPER)
    in_maps = []
    for c in range(NCORES):
        m = {}
        off = 0
        for i, fsz in enumerate(TILE_SIZES):
            n = P * fsz
            m[f"xs{i}"] = shards[c, off : off + n].reshape(P, fsz)
            off += n
        in_maps.append(m)
    return in_maps


def kernel(Xs: np.ndarray, bins: np.ndarray) -> np.ndarray:
    Xs = np.asarray(Xs, dtype=np.float32)
    bins = np.asarray(bins, dtype=np.float32)
    nc = build(bins)
    res = run_bass_kernel_spmd(nc, make_in_maps(Xs), core_ids=list(range(NCORES)))
    out = np.concatenate(
        [
            np.concatenate([r[f"out{i}"].reshape(-1) for i in range(len(TILE_SIZES))])
            for r in res.results
        ]
    )
    return out.astype(np.float32)



# revision 2
# speedup vs baseline: 1.0535x; 1.0535x over previous
"""Trainium2 Bass kernel for nn_LogOddsPerformanceTransformer.

Computes, for each element x of Xs:
    s   = log(x) - log(1-x)              (log-odds)
    idx = clip(searchsorted(bins, max(s, bins[0]), 'right') - 1, 0, NB-1)
    out = bins[idx]

bins is a uniform grid (linspace), so binning reduces to an affine floor
via the magic-number rounding trick.  The whole post-log chain runs in
TWO fused custom-DVE ops (4 ALU stages each, single DVE instruction
cost per tile):

    OP1:  sg  = clip((a - b) * inv, -31.0, 32.0)     # sigma = s/step - shift
    OP2:  out = (((sg + M) - M) - 0.5) * step        # M = 2^23+31 magic floor

Clamp bounds -31/32 (instead of the exact -31.5/32.5 bin edges) keep
sg + 31 >= 0 so the magic add always lands on the integer rounding grid
at 2^23; any clamp value inside the first/last bin gives the identical
bin index.

Data parallel over 8 NeuronCores; per core the 524288-element slice is
processed as a sequence of [128 x fsz] tiles.  Input DMAs issue on the
SP sequencer (HWDGE), output DMAs on gpsimd (SWDGE) so the two issue
paths don't contend; compute is ACT (2x Ln) + DVE (2 fused ops).
"""

import sys

sys.path.insert(0, "/opt/trn_rl_repo")

from contextlib import ExitStack

import numpy as np

import concourse.bass as bass
import concourse.tile as tile
from concourse import bacc, mybir
from concourse.bass_utils import run_bass_kernel_spmd

N = 4_194_304
NCORES = 8
NPER = N // NCORES  # 524288
P = 128

# --- tunables -------------------------------------------------------------
TILE_SIZES = (256, 512, 768, 768, 768, 768, 256)  # sum = 4096
OUT_DMA_ENGINE = "gpsimd"  # swdge path; frees HWDGE for input DMAs
# --------------------------------------------------------------------------

f32 = mybir.dt.float32
Alu = mybir.AluOpType
Act = mybir.ActivationFunctionType

_BUILD_CACHE: dict[tuple, object] = {}


# --- custom DVE ops -------------------------------------------------------
def _sigma_ref(in0, in1, s0, s1, imm2):
    f = np.float32
    d = (in0.astype(f) - in1.astype(f)).astype(f)
    sg = (d * f(s0)).astype(f)
    return np.minimum(np.maximum(sg, f(s1)), f(imm2)).astype(f)


def _bin_ref(in0, in1, s0, s1, imm2):
    f = np.float32
    t1 = (in0.astype(f) + f(s0)).astype(f)
    j = (t1 - f(s0)).astype(f)
    jsh = (j - f(s1)).astype(f)
    return (jsh * f(imm2)).astype(f)


def _register_ops():
    import concourse.dve_ops as dve_ops
    from concourse.dve_spec import (
        Spec,
        Src0,
        Src1,
        C0,
        C1,
        C2,
        maxx,
        minn,
        lower,
        _has_src1,
    )
    from concourse.dve_uop import DveOpSpec

    def reg(name, spec):
        if name in dve_ops._SUB_OPCODE_FOR_NAME:
            return next(op for op in dve_ops.OPS if op.name == name)
        row = max(dve_ops._SUB_OPCODE_FOR_NAME.values()) + 1
        assert row < 0x20
        dve_ops._SUB_OPCODE_FOR_NAME[name] = row
        shas = {}
        for ver in ("v3", "v4"):
            uops = lower(spec, ver=ver)
            shas[ver] = DveOpSpec(
                name=name, opcode=row, uops=uops, rd1_en=_has_src1(spec)
            ).sha(ver)
        op = dve_ops.DveOp(name, spec, subdim=False, uops_sha=shas)
        dve_ops.OPS.append(op)
        dve_ops.CUSTOM_DVE_SPECS[name] = spec
        return op

    op1 = reg(
        "LOGODDS_SIGMA_ANT",
        Spec(body=minn(maxx((Src0 - Src1) * C0, C1), C2), reference=_sigma_ref),
    )
    op2 = reg(
        "LOGODDS_MAGICBIN_ANT",
        Spec(body=(((Src0 + C0) - C0) - C1) * C2, reference=_bin_ref),
    )
    return op1, op2


_OP1, _OP2 = _register_ops()


def _constants(bins: np.ndarray):
    """Host-side constants; returns None if the fused path can't be used
    (non-uniform bins or grid where the magic offsets aren't exact)."""
    b64 = bins.astype(np.float64)
    nb = len(bins)
    if nb != 64:
        return None
    step = np.float32((b64[-1] - b64[0]) / (nb - 1))
    inv = np.float32((nb - 1) / (b64[-1] - b64[0]))
    # sigma = s*inv ; bin edges at sigma = b0*inv + k.  Require b0*inv = -31.5
    # (true for the symmetric linspace(-6,6,64) grid) so the fixed clamp
    # bounds/magic below are exact.
    if not np.isclose(float(b64[0]) * float(inv), -31.5, atol=1e-6):
        return None
    uniform = np.allclose(
        np.diff(b64), (b64[-1] - b64[0]) / (nb - 1), rtol=0, atol=1e-5
    )
    if not uniform:
        return None
    return (float(inv), float(step))


MAGIC = float(np.float32(2.0**23 + 31.0))
SIG_LO = -31.0
SIG_HI = 32.0
HALF = 0.5


def _build(inv, step):
    assert sum(TILE_SIZES) * P == NPER
    nc = bacc.Bacc("TRN2", target_bir_lowering=False, debug=False)
    xs = [
        nc.dram_tensor(f"xs{i}", [P, fsz], f32, kind="ExternalInput").ap()
        for i, fsz in enumerate(TILE_SIZES)
    ]
    outs = [
        nc.dram_tensor(f"out{i}", [P, fsz], f32, kind="ExternalOutput").ap()
        for i, fsz in enumerate(TILE_SIZES)
    ]

    with tile.TileContext(nc) as tc, ExitStack() as ctx:
        xpool = ctx.enter_context(tc.tile_pool(name="xpool", bufs=1))
        tmp = ctx.enter_context(tc.tile_pool(name="tmp", bufs=1))
        dma_out = getattr(nc, OUT_DMA_ENGINE)

        # all input DMAs issued first (high priority) so the out DMAs never
        # starve later input tiles
        x_tiles = []
        with tc.high_priority():
            for i, fsz in enumerate(TILE_SIZES):
                x = xpool.tile([P, fsz], f32, tag=f"x{i}")
                nc.sync.dma_start(x[:], xs[i][:])
                x_tiles.append(x)

        for i, fsz in enumerate(TILE_SIZES):
            x = x_tiles[i]
            a = tmp.tile([P, fsz], f32, tag=f"a{i}")
            b = tmp.tile([P, fsz], f32, tag=f"b{i}")
            nc.scalar.activation(a[:], x[:], Act.Ln)
            nc.scalar.activation(b[:], x[:], Act.Ln, 1.0, -1.0)
            sg = tmp.tile([P, fsz], f32, tag=f"s{i}")
            nc.vector._custom_dve(
                _OP1, out=sg[:], in0=a[:], in1=b[:], s0=inv, s1=SIG_LO, imm2=SIG_HI
            )
            o = tmp.tile([P, fsz], f32, tag=f"o{i}")
            nc.vector._custom_dve(
                _OP2, out=o[:], in0=sg[:], s0=MAGIC, s1=HALF, imm2=step
            )
            dma_out.dma_start(outs[i][:], o[:])

    nc.compile()
    return nc


def build(bins: np.ndarray):
    key = _constants(bins)
    if key is None:
        raise NotImplementedError("unsupported bins for this kernel")
    if key not in _BUILD_CACHE:
        _BUILD_CACHE[key] = _build(*key)
    return _BUILD_CACHE[key]


def make_in_maps(Xs: np.ndarray):
    shards = Xs.reshape(NCORES, NPER)
    in_maps = []
    for c in range(NCORES):
        m = {}
        off = 0
        for i, fsz in enumerate(TILE_SIZES):
            n = P * fsz
            m[f"xs{i}"] = shards[c, off : off + n].reshape(P, fsz)
            off += n
        in_maps.append(m)
    return in_maps


def kernel(Xs: np.ndarray, bins: np.ndarray) -> np.ndarray:
    Xs = np.asarray(Xs, dtype=np.float32)
    bins = np.asarray(bins, dtype=np.float32)
    nc = build(bins)
    res = run_bass_kernel_spmd(nc, make_in_maps(Xs), core_ids=list(range(NCORES)))
    out = np.concatenate(
        [
            np.concatenate([r[f"out{i}"].reshape(-1) for i in range(len(TILE_SIZES))])
            for r in res.results
        ]
    )
    return out.astype(np.float32)


# revision 17
# speedup vs baseline: 1.1532x; 1.0946x over previous
"""Trainium2 Bass kernel for nn_LogOddsPerformanceTransformer.

Computes, for each element x of Xs:
    s   = log(x) - log(1-x)              (log-odds)
    idx = clip(searchsorted(bins, max(s, bins[0]), 'right') - 1, 0, NB-1)
    out = bins[idx]

bins is a uniform grid (linspace), so binning reduces to an affine floor
via the magic-number rounding trick.  The post-log chain is one fused
6-stage custom-DVE op producing the integer bin offset j, plus one
2-ALU tensor_scalar (on gpsimd) for the final affine:

    OP1:  j   = ((clip((a-b)*inv, -31, 32) + M) - M)   # M = 2^23+31
    TS:   out = (j - 0.5) * step

Clamp bounds -31/32 (instead of the exact bin edges -31.5/32.5) keep
sg + 31 >= 0 so the magic add always lands on the integer rounding grid
at 2^23; any clamp value inside the first/last bin gives the identical
bin index.

Data parallel over 8 NeuronCores; per core the 524288-element slice is
viewed as [128 x 4096].  Input DMAs (SP/HWDGE) use a ramped column-tile
grid so the activation engine is never starved; compute runs on an
independent column-chunk grid (ACT 2x Ln -> DVE fused op -> Pool TS);
output DMAs issue per compute chunk on the SP sequencer, which is idle
after the input DMAs and whose in-order semaphore waits match the chunk
completion order.
"""

import sys

sys.path.insert(0, "/opt/trn_rl_repo")

from contextlib import ExitStack

import numpy as np

import concourse.bass as bass
import concourse.tile as tile
from concourse import bacc, mybir
from concourse.bass_utils import run_bass_kernel_spmd

N = 4_194_304
NCORES = 8
NPER = N // NCORES  # 524288
P = 128
W = NPER // P  # 4096 columns per core

# --- tunables -------------------------------------------------------------
IN_TILES = (128, 256, 512, 896, 1152, 1152)  # ramped; sum = 4096
# compute grid; sum = 4096.  Chunk ends should align under tile prefix sums
# so a chunk never waits on a tile it doesn't cover.
CHUNKS = (128, 256, 512, 896, 576, 576, 576, 448, 128)
# out-DMA grid; boundaries must be a subset of the chunk prefix sums.
OUT_TILES = CHUNKS
OUT_ENGINES = None  # per-out issue engine names; None -> all "sync"
TAIL_TS_ON_DVE = 99  # last k chunks run the final tensor_scalar on DVE not Pool
# --------------------------------------------------------------------------

f32 = mybir.dt.float32
Alu = mybir.AluOpType
Act = mybir.ActivationFunctionType

_BUILD_CACHE: dict[tuple, object] = {}


# --- custom DVE op --------------------------------------------------------
def _j_ref(in0, in1, s0, s1, imm2):
    f = np.float32
    d = (in0.astype(f) - in1.astype(f)).astype(f)
    sg = (d * f(s0)).astype(f)
    mx = np.maximum(sg, f(imm2)).astype(f)
    lat = f(f(1.0) - f(imm2))
    mn = np.minimum(mx, lat).astype(f)
    t1 = (mn + f(s1)).astype(f)
    return (t1 - f(s1)).astype(f)


def _register_ops():
    import concourse.dve_ops as dve_ops
    from concourse.dve_spec import (
        Spec,
        Src0,
        Src1,
        C0,
        C1,
        C2,
        One,
        maxx,
        minn,
        lower,
        _has_src1,
    )
    from concourse.dve_uop import DveOpSpec

    def reg(name, spec):
        if name in dve_ops._SUB_OPCODE_FOR_NAME:
            return next(op for op in dve_ops.OPS if op.name == name)
        row = max(dve_ops._SUB_OPCODE_FOR_NAME.values()) + 1
        assert row < 0x20
        dve_ops._SUB_OPCODE_FOR_NAME[name] = row
        shas = {}
        for ver in ("v3", "v4"):
            uops = lower(spec, ver=ver)
            shas[ver] = DveOpSpec(
                name=name, opcode=row, uops=uops, rd1_en=_has_src1(spec)
            ).sha(ver)
        op = dve_ops.DveOp(name, spec, subdim=False, uops_sha=shas)
        dve_ops.OPS.append(op)
        dve_ops.CUSTOM_DVE_SPECS[name] = spec
        return op

    # j = ((clip((a-b)*C0, C2, 1-C2) + C1) - C1);  C2 = -31 so 1-C2 = 32
    sg = (Src0 - Src1) * C0
    mn = minn(maxx(sg, C2), One - C2)
    body = (mn + C1) - C1
    return reg("LOGODDS_J_ANT", Spec(body=body, reference=_j_ref))


_OP1 = _register_ops()


def _constants(bins: np.ndarray):
    """Host-side constants; returns None if the fused path can't be used
    (non-uniform bins or grid where the magic offsets aren't exact)."""
    b64 = bins.astype(np.float64)
    nb = len(bins)
    if nb != 64:
        return None
    step = np.float32((b64[-1] - b64[0]) / (nb - 1))
    inv = np.float32((nb - 1) / (b64[-1] - b64[0]))
    # sigma = s*inv ; bin edges at sigma = b0*inv + k.  Require b0*inv = -31.5
    # (true for the symmetric linspace(-6,6,64) grid) so the fixed clamp
    # bounds/magic below are exact.
    if not np.isclose(float(b64[0]) * float(inv), -31.5, atol=1e-6):
        return None
    uniform = np.allclose(
        np.diff(b64), (b64[-1] - b64[0]) / (nb - 1), rtol=0, atol=1e-5
    )
    if not uniform:
        return None
    return (float(inv), float(step))


MAGIC = float(np.float32(2.0**23 + 31.0))
SIG_LO = -31.0  # imm2 of OP1; upper clamp is 1-imm2 = 32
HALF = 0.5


def _build(inv, step):
    assert sum(IN_TILES) == W and sum(CHUNKS) == W and sum(OUT_TILES) == W
    ccum = np.cumsum(CHUNKS)
    assert set(np.cumsum(OUT_TILES)) <= set(ccum), "OUT_TILES must nest in CHUNKS"

    # Route the framework's preamble const-AP memsets (gpsimd) to the DVE
    # engine: the 4 Pool memsets otherwise delay the kernel start barrier by
    # ~0.5us (GPSIMD Q7 launch overhead per memset).
    import concourse.bass as _bass_mod

    _orig_memset = _bass_mod.BassSharedVectorInterface.memset

    def _memset_on_dve(self, ap, constant):
        eng = getattr(self, "engine", None)
        b = getattr(self, "bass", None)
        if (
            eng == mybir.EngineType.Pool
            and b is not None
            and getattr(b, "vector", None) is not None
        ):
            return _orig_memset(b.vector, ap, constant)
        return _orig_memset(self, ap, constant)

    _bass_mod.BassSharedVectorInterface.memset = _memset_on_dve
    try:
        nc = bacc.Bacc("TRN2", target_bir_lowering=False, debug=False)
    finally:
        _bass_mod.BassSharedVectorInterface.memset = _orig_memset
    xs = nc.dram_tensor("xs", [P, W], f32, kind="ExternalInput").ap()
    outs = nc.dram_tensor("out", [P, W], f32, kind="ExternalOutput").ap()

    with tile.TileContext(nc) as tc, ExitStack() as ctx:
        tmp = ctx.enter_context(tc.tile_pool(name="tmp", bufs=1))

        x = tmp.tile([P, W], f32, tag="x")
        a = tmp.tile([P, W], f32, tag="a")
        b = tmp.tile([P, W], f32, tag="b")
        j = tmp.tile([P, W], f32, tag="j")
        o = tmp.tile([P, W], f32, tag="o")

        # all input DMAs issued first (high priority) so the out DMAs never
        # starve later input tiles
        with tc.high_priority():
            off = 0
            for w in IN_TILES:
                sl = (slice(None), slice(off, off + w))
                nc.sync.dma_start(x[sl], xs[sl])
                off += w

        # scalar constants built with DVE memsets (idle engine) so no
        # const-pool Memset gates the start barrier
        bias0 = tmp.tile([P, 1], f32, tag="bias0")
        bias1 = tmp.tile([P, 1], f32, tag="bias1")
        half_ap = tmp.tile([P, 1], f32, tag="half")
        step_ap = tmp.tile([P, 1], f32, tag="step")
        nc.vector.memset(bias0[:], 0.0)
        nc.vector.memset(bias1[:], 1.0)
        nc.vector.memset(half_ap[:], HALF)
        nc.vector.memset(step_ap[:], step)
        # warmup: forces the Ln act-table load to run during the DMA ramp
        # instead of gating the first real activation
        warm = tmp.tile([P, 1], f32, tag="warm")
        nc.scalar.activation(warm[:], bias1[:], Act.Ln, bias0[:])

        NCH = len(CHUNKS)
        out_cum = list(np.cumsum(OUT_TILES))
        off = 0
        for ci, w in enumerate(CHUNKS):
            sl = (slice(None), slice(off, off + w))
            nc.scalar.activation(a[sl], x[sl], Act.Ln, bias0[:])
            nc.scalar.activation(b[sl], x[sl], Act.Ln, bias1[:], -1.0)
            nc.vector._custom_dve(
                _OP1, out=j[sl], in0=a[sl], in1=b[sl], s0=inv, s1=MAGIC, imm2=SIG_LO
            )
            ts_eng = nc.vector if ci >= NCH - TAIL_TS_ON_DVE else nc.gpsimd
            ts_eng.tensor_scalar(
                o[sl], j[sl], half_ap[:], step_ap[:], Alu.subtract, Alu.mult
            )
            off += w
            if off in out_cum:
                oi = out_cum.index(off)
                prev = 0 if oi == 0 else out_cum[oi - 1]
                osl = (slice(None), slice(prev, off))
                eng = "sync" if OUT_ENGINES is None else OUT_ENGINES[oi]
                getattr(nc, eng).dma_start(outs[osl], o[osl])

    nc.compile()
    return nc


def build(bins: np.ndarray):
    key = _constants(bins)
    if key is None:
        raise NotImplementedError("unsupported bins for this kernel")
    if key not in _BUILD_CACHE:
        _BUILD_CACHE[key] = _build(*key)
    return _BUILD_CACHE[key]


def make_in_maps(Xs: np.ndarray):
    shards = Xs.reshape(NCORES, P, W)
    return [{"xs": shards[c]} for c in range(NCORES)]


def kernel(Xs: np.ndarray, bins: np.ndarray) -> np.ndarray:
    Xs = np.asarray(Xs, dtype=np.float32)
    bins = np.asarray(bins, dtype=np.float32)
    nc = build(bins)
    res = run_bass_kernel_spmd(nc, make_in_maps(Xs), core_ids=list(range(NCORES)))
    out = np.concatenate([r["out"].reshape(-1) for r in res.results])
    return out.astype(np.float32)


# revision 23
# speedup vs baseline: 1.1775x; 1.0211x over previous
"""Trainium2 Bass kernel for nn_LogOddsPerformanceTransformer.

Computes, for each element x of Xs:
    s   = log(x) - log(1-x)              (log-odds)
    idx = clip(searchsorted(bins, max(s, bins[0]), 'right') - 1, 0, NB-1)
    out = bins[idx]

bins is a uniform grid (linspace), so binning reduces to an affine floor
via the magic-number rounding trick.  The post-log chain is one fused
6-stage custom-DVE op producing the integer bin offset j, plus one
2-ALU tensor_scalar (on gpsimd) for the final affine:

    OP1:  j   = ((clip((a-b)*inv, -31, 32) + M) - M)   # M = 2^23+31
    TS:   out = (j - 0.5) * step

Clamp bounds -31/32 (instead of the exact bin edges -31.5/32.5) keep
sg + 31 >= 0 so the magic add always lands on the integer rounding grid
at 2^23; any clamp value inside the first/last bin gives the identical
bin index.

Data parallel over 8 NeuronCores; per core the 524288-element slice is
viewed as [128 x 4096].  Input DMAs (SP/HWDGE) use a ramped column-tile
grid so the activation engine is never starved; compute runs on an
independent column-chunk grid (ACT 2x Ln -> DVE fused op -> Pool TS);
output DMAs issue per compute chunk on the SP sequencer, which is idle
after the input DMAs and whose in-order semaphore waits match the chunk
completion order.
"""

import sys

sys.path.insert(0, "/opt/trn_rl_repo")

from contextlib import ExitStack

import numpy as np

import concourse.bass as bass
import concourse.tile as tile
from concourse import bacc, mybir
from concourse.bass_utils import run_bass_kernel_spmd

N = 4_194_304
NCORES = 8
NPER = N // NCORES  # 524288
P = 128
W = NPER // P  # 4096 columns per core

# --- tunables -------------------------------------------------------------
IN_TILES = (128, 256, 512, 896, 1152, 1152)  # ramped; sum = 4096
# compute grid; sum = 4096.  Chunk ends should align under tile prefix sums
# so a chunk never waits on a tile it doesn't cover.
CHUNKS = (128, 256, 512, 896, 640, 640, 512, 384, 128)
# out-DMA grid; boundaries must be a subset of the chunk prefix sums.
OUT_TILES = CHUNKS
# per-out issue engine names; None -> all "sync".  The second-to-last out on
# "scalar" (ACT sequencer, idle by then) overlaps the last one's SP issue.
OUT_ENGINES = ("sync",) * 7 + ("scalar", "sync")
TAIL_TS_ON_DVE = 99  # last k chunks run the final tensor_scalar on DVE not Pool
TAIL_HIPRI = 0  # last k chunks emit OP1+TS under tc.high_priority()
# --------------------------------------------------------------------------

f32 = mybir.dt.float32
Alu = mybir.AluOpType
Act = mybir.ActivationFunctionType

_BUILD_CACHE: dict[tuple, object] = {}


# --- custom DVE op --------------------------------------------------------
def _j_ref(in0, in1, s0, s1, imm2):
    f = np.float32
    d = (in0.astype(f) - in1.astype(f)).astype(f)
    sg = (d * f(s0)).astype(f)
    mx = np.maximum(sg, f(imm2)).astype(f)
    lat = f(f(1.0) - f(imm2))
    mn = np.minimum(mx, lat).astype(f)
    t1 = (mn + f(s1)).astype(f)
    return (t1 - f(s1)).astype(f)


def _register_ops():
    import concourse.dve_ops as dve_ops
    from concourse.dve_spec import (
        Spec,
        Src0,
        Src1,
        C0,
        C1,
        C2,
        One,
        maxx,
        minn,
        lower,
        _has_src1,
    )
    from concourse.dve_uop import DveOpSpec

    def reg(name, spec):
        if name in dve_ops._SUB_OPCODE_FOR_NAME:
            return next(op for op in dve_ops.OPS if op.name == name)
        row = max(dve_ops._SUB_OPCODE_FOR_NAME.values()) + 1
        assert row < 0x20
        dve_ops._SUB_OPCODE_FOR_NAME[name] = row
        shas = {}
        for ver in ("v3", "v4"):
            uops = lower(spec, ver=ver)
            shas[ver] = DveOpSpec(
                name=name, opcode=row, uops=uops, rd1_en=_has_src1(spec)
            ).sha(ver)
        op = dve_ops.DveOp(name, spec, subdim=False, uops_sha=shas)
        dve_ops.OPS.append(op)
        dve_ops.CUSTOM_DVE_SPECS[name] = spec
        return op

    # j = ((clip((a-b)*C0, C2, 1-C2) + C1) - C1);  C2 = -31 so 1-C2 = 32
    sg = (Src0 - Src1) * C0
    mn = minn(maxx(sg, C2), One - C2)
    body = (mn + C1) - C1
    return reg("LOGODDS_J_ANT", Spec(body=body, reference=_j_ref))


_OP1 = _register_ops()


def _constants(bins: np.ndarray):
    """Host-side constants; returns None if the fused path can't be used
    (non-uniform bins or grid where the magic offsets aren't exact)."""
    b64 = bins.astype(np.float64)
    nb = len(bins)
    if nb != 64:
        return None
    step = np.float32((b64[-1] - b64[0]) / (nb - 1))
    inv = np.float32((nb - 1) / (b64[-1] - b64[0]))
    # sigma = s*inv ; bin edges at sigma = b0*inv + k.  Require b0*inv = -31.5
    # (true for the symmetric linspace(-6,6,64) grid) so the fixed clamp
    # bounds/magic below are exact.
    if not np.isclose(float(b64[0]) * float(inv), -31.5, atol=1e-6):
        return None
    uniform = np.allclose(
        np.diff(b64), (b64[-1] - b64[0]) / (nb - 1), rtol=0, atol=1e-5
    )
    if not uniform:
        return None
    return (float(inv), float(step))


MAGIC = float(np.float32(2.0**23 + 31.0))
SIG_LO = -31.0  # imm2 of OP1; upper clamp is 1-imm2 = 32
HALF = 0.5


def _build(inv, step):
    assert sum(IN_TILES) == W and sum(CHUNKS) == W and sum(OUT_TILES) == W
    ccum = np.cumsum(CHUNKS)
    assert set(np.cumsum(OUT_TILES)) <= set(ccum), "OUT_TILES must nest in CHUNKS"

    return _build_body(inv, step)


def _retag_const_memsets(nc):
    """Move the framework preamble's const-AP memsets off the Pool engine:
    their GPSIMD Q7 launch overhead (95ns each, serialized) otherwise delays
    the kernel start barrier by ~0.5us.  The all-engine barrier drains every
    engine, so the memsets complete before any consumer regardless of
    engine."""
    for bb in nc.m.functions[0].blocks:
        for ins in bb.instructions:
            if (
                type(ins).__name__ == "InstMemset"
                and ins.engine == mybir.EngineType.Pool
            ):
                ins.engine = mybir.EngineType.DVE


def _build_body(inv, step):
    nc = bacc.Bacc("TRN2", target_bir_lowering=False, debug=False)
    xs = nc.dram_tensor("xs", [P, W], f32, kind="ExternalInput").ap()
    outs = nc.dram_tensor("out", [P, W], f32, kind="ExternalOutput").ap()

    with tile.TileContext(nc) as tc, ExitStack() as ctx:
        tmp = ctx.enter_context(tc.tile_pool(name="tmp", bufs=1))

        x = tmp.tile([P, W], f32, tag="x")
        a = tmp.tile([P, W], f32, tag="a")
        b = tmp.tile([P, W], f32, tag="b")
        j = tmp.tile([P, W], f32, tag="j")
        o = tmp.tile([P, W], f32, tag="o")

        # all input DMAs issued first (high priority) so the out DMAs never
        # starve later input tiles
        with tc.high_priority():
            off = 0
            for w in IN_TILES:
                sl = (slice(None), slice(off, off + w))
                nc.sync.dma_start(x[sl], xs[sl])
                off += w

        # scalar constants built with DVE memsets (idle engine) so no
        # const-pool Memset gates the start barrier
        bias0 = tmp.tile([P, 1], f32, tag="bias0")
        bias1 = tmp.tile([P, 1], f32, tag="bias1")
        half_ap = tmp.tile([P, 1], f32, tag="half")
        step_ap = tmp.tile([P, 1], f32, tag="step")
        nc.vector.memset(bias0[:], 0.0)
        nc.vector.memset(bias1[:], 1.0)
        nc.vector.memset(half_ap[:], HALF)
        nc.vector.memset(step_ap[:], step)
        # warmup: forces the Ln act-table load to run during the DMA ramp
        # instead of gating the first real activation
        warm = tmp.tile([P, 1], f32, tag="warm")
        nc.scalar.activation(warm[:], bias1[:], Act.Ln, bias0[:])

        NCH = len(CHUNKS)
        out_cum = list(np.cumsum(OUT_TILES))
        off = 0
        for ci, w in enumerate(CHUNKS):
            sl = (slice(None), slice(off, off + w))
            nc.scalar.activation(a[sl], x[sl], Act.Ln, bias0[:])
            nc.scalar.activation(b[sl], x[sl], Act.Ln, bias1[:], -1.0)
            hipri = tc.high_priority() if ci >= NCH - TAIL_HIPRI else None
            if hipri is not None:
                hipri.__enter__()
            nc.vector._custom_dve(
                _OP1, out=j[sl], in0=a[sl], in1=b[sl], s0=inv, s1=MAGIC, imm2=SIG_LO
            )
            ts_eng = nc.vector if ci >= NCH - TAIL_TS_ON_DVE else nc.gpsimd
            ts_eng.tensor_scalar(
                o[sl], j[sl], half_ap[:], step_ap[:], Alu.subtract, Alu.mult
            )
            if hipri is not None:
                hipri.__exit__(None, None, None)
            off += w
            if off in out_cum:
                oi = out_cum.index(off)
                prev = 0 if oi == 0 else out_cum[oi - 1]
                osl = (slice(None), slice(prev, off))
                eng = "sync" if OUT_ENGINES is None else OUT_ENGINES[oi]
                getattr(nc, eng).dma_start(outs[osl], o[osl])

    _retag_const_memsets(nc)
    nc.compile()
    return nc


def build(bins: np.ndarray):
    key = _constants(bins)
    if key is None:
        raise NotImplementedError("unsupported bins for this kernel")
    if key not in _BUILD_CACHE:
        _BUILD_CACHE[key] = _build(*key)
    return _BUILD_CACHE[key]


def make_in_maps(Xs: np.ndarray):
    shards = Xs.reshape(NCORES, P, W)
    return [{"xs": shards[c]} for c in range(NCORES)]


def kernel(Xs: np.ndarray, bins: np.ndarray) -> np.ndarray:
    Xs = np.asarray(Xs, dtype=np.float32)
    bins = np.asarray(bins, dtype=np.float32)
    nc = build(bins)
    res = run_bass_kernel_spmd(nc, make_in_maps(Xs), core_ids=list(range(NCORES)))
    out = np.concatenate([r["out"].reshape(-1) for r in res.results])
    return out.astype(np.float32)
